# revision 11
# baseline (speedup 1.0000x reference)
"""GCC-PHAT Trainium2 kernel (v7: v2 skeleton + uniform 12-plane inverse).

Pipeline (per core, batch-sharded B=16 -> 2 per core):
  1. Forward rfft as PE matmul in fp16 (fp32 PSUM accumulate):
     xT[b,m,n,t] (host-pretransposed, fp16) @ F[1024,1024] fp16.
     F cols 0..511 = cos(2pi n f/L) f=1..512; cols 512..1022 = -sin, f=1..511;
     col 1023 = zeros (Im X[512] = 0).  Output X.T in PSUM, f on partitions
     (8 chunks of 128: 4 'a' = Re f=128c+r+1, 4 'b' = Im).
  2. PHAT normalize per mic (uniform, no special rows): w' =
     1/sqrt(16*(a^2+b^2)) via ACT Abs_reciprocal_sqrt; ya = a*w', yb = b*w'
     (unit/4) fp16; ys = ya+yb, yd = ya-yb.
     Bin 0 (DC) is handled on the host: PHAT reduces it to
     sign(S1)*sign(S2)/L, constant over lags.  f=512 (chunk3 row127)
     works naturally: its sin rows are identically zero.
  3. Pair products (28 mic pairs, diagonal pairing): Karatsuba
     k1 = ys1*a2, k2 = a1*ys2, k3 = b1*yd2 -> 12 planes fp16
     (DVE: k1,k2,k3[c=3]; Pool: k3[c<3]).
  4. Truncated inverse DFT as PE matmul, G stationary: 12 K-chunks of
     [128f x 64 lags] accumulated into PSUM [64, rows].  G rows carry 16x
     scale + irfft weights/fftshift/slice.
  5. PSUM -> ACT copy -> SBUF -> DMA to out[b, lag, p, t] (lag-major;
     host transposes back and adds the DC term).
  Input DMAs ride the ACT HWDGE queue so the (late-bound) output DMAs on
  sync never head-of-line block the next batch's input.
"""

import os
from contextlib import ExitStack

import numpy as np

import concourse.bass as bass
import concourse.bacc as bacc
import concourse.mybir as mybir
import concourse.tile as tile
from concourse.bass import ds, ts
from concourse.bass_utils import run_bass_kernel_spmd

B, M, T, L = 16, 8, 250, 1024
NCORES = 8
NB = B // NCORES          # batches per core
NPAIRS = (M * (M - 1)) // 2   # 28
NL = 64                   # output lags
F32 = mybir.dt.float32
FP16 = mybir.dt.float16


def _build_F() -> np.ndarray:
    n = np.arange(L, dtype=np.float64)[:, None]
    F = np.zeros((L, L), dtype=np.float64)
    f_a = np.arange(1, 513, dtype=np.float64)[None, :]
    f_b = np.arange(1, 512, dtype=np.float64)[None, :]
    F[:, 0:512] = np.cos(2 * np.pi * n * f_a / L)
    F[:, 512:1023] = -np.sin(2 * np.pi * n * f_b / L)
    F[:, 1023] = 0.0            # Im X[512] slot
    return F.astype(np.float16)


def _build_G() -> np.ndarray:
    """12 inverse planes [128, 64]: idx c = k1, 4+c = k2, 8+c = k3."""
    G = np.zeros((12, 128, NL), dtype=np.float64)
    nj = (np.arange(NL) - 32).astype(np.float64)
    for c in range(4):
        for r in range(128):
            f = 128 * c + r + 1
            w = 1.0 if f == 512 else 2.0
            cosv = 16.0 * w * np.cos(2 * np.pi * f * nj / L) / L
            sinv = 16.0 * w * np.sin(2 * np.pi * f * nj / L) / L
            G[0 + c, r] = cosv - sinv     # k1 = ys1*a2
            G[4 + c, r] = sinv            # k2 = a1*ys2
            G[8 + c, r] = -cosv           # k3 = b1*yd2
    return G.astype(np.float16)


def build_bass() -> bass.Bass:
    nc = bacc.Bacc("TRN2", target_bir_lowering=False, debug=False)
    xT = nc.dram_tensor("xT", [NB, M, L, T], FP16, kind="ExternalInput")
    out = nc.dram_tensor("out", [NB, NL, NPAIRS, T], F32, kind="ExternalOutput")
    Fh = nc.inline_tensor(_build_F(), name="Fmat")
    Gh = nc.inline_tensor(np.ascontiguousarray(_build_G()), name="Gmat")

    with tile.TileContext(nc) as tc, ExitStack() as ctx:
        consts = ctx.enter_context(tc.tile_pool(name="consts", bufs=1))
        xt_pool = ctx.enter_context(tc.tile_pool(name="xt", bufs=2))
        # y double-buffered: batch b+1's forward/normalize overlaps batch b's
        # pairs+inverse phase
        y_pool = ctx.enter_context(tc.tile_pool(name="y", bufs=2))
        tmp_pool = ctx.enter_context(tc.tile_pool(name="tmp", bufs=2))
        r_pool = ctx.enter_context(tc.tile_pool(name="r", bufs=2))
        fwd_psum = ctx.enter_context(tc.tile_pool(name="fps", bufs=3, space="PSUM"))
        inv_psum = ctx.enter_context(tc.tile_pool(name="ips", bufs=2, space="PSUM"))

        f_sb = consts.tile([128, 8, L], FP16)
        fr = Fh[:].rearrange("(k p) c -> p k c", p=128)
        for k in range(8):
            # split so the first matmuls only wait on the first 256KB chunk
            nc.sync.dma_start(f_sb[:, k], fr[:, k])
        g_sb = consts.tile([128, 12, NL], FP16)
        nc.sync.dma_start(g_sb[:], Gh[:].rearrange("i p j -> p i j"))

        for b in range(NB):
            # Y tiles: [128, mg(4), m(2), t] fp16 per (chunk, plane)
            ya = [y_pool.tile([128, 4, 2, T], FP16, tag=f"ya{c}", name=f"ya{c}") for c in range(4)]
            yb = [y_pool.tile([128, 4, 2, T], FP16, tag=f"yb{c}", name=f"yb{c}") for c in range(4)]
            ys = [y_pool.tile([128, 4, 2, T], FP16, tag=f"ys{c}", name=f"ys{c}") for c in range(4)]
            yd = [y_pool.tile([128, 4, 2, T], FP16, tag=f"yd{c}", name=f"yd{c}") for c in range(4)]

            # ---- forward + normalize ----
            for mg in range(4):
                xt_sb = xt_pool.tile([128, 8, 2, T], FP16, tag="xt")
                for mi in range(2):
                    nc.scalar.dma_start(
                        xt_sb[:, :, mi],
                        xT[b, 2 * mg + mi].rearrange("(k p) t -> p k t", p=128),
                    )
                for c in range(4):
                    ps_a = fwd_psum.tile([128, 2, T], F32, tag="psa")
                    ps_b = fwd_psum.tile([128, 2, T], F32, tag="psb")
                    for k in range(8):
                        nc.tensor.matmul(
                            ps_a[:],
                            f_sb[:, k, ts(c, 128)],
                            xt_sb[:, k],
                            start=(k == 0), stop=(k == 7),
                        )
                    for k in range(8):
                        nc.tensor.matmul(
                            ps_b[:],
                            f_sb[:, k, ts(4 + c, 128)],
                            xt_sb[:, k],
                            start=(k == 0), stop=(k == 7),
                        )
                    # normalize (uniform across all rows)
                    sq_a = tmp_pool.tile([128, 2, T], F32, tag="sqa")
                    sq_b = tmp_pool.tile([128, 2, T], F32, tag="sqb")
                    w = tmp_pool.tile([128, 2, T], F32, tag="w")
                    nc.scalar.square(sq_a[:], ps_a[:])
                    nc.scalar.square(sq_b[:], ps_b[:])
                    nc.gpsimd.tensor_add(sq_a[:], sq_a[:], sq_b[:])
                    # w' = 1/sqrt(16*r) = (1/|X|)/4
                    nc.scalar.activation(
                        w[:], sq_a[:],
                        mybir.ActivationFunctionType.Abs_reciprocal_sqrt,
                        scale=16.0,
                    )
                    nc.vector.tensor_mul(ya[c][:, mg], ps_a[:], w[:])
                    nc.vector.tensor_mul(yb[c][:, mg], ps_b[:], w[:])
                    nc.vector.tensor_add(ys[c][:, mg], ya[c][:, mg], yb[c][:, mg])
                    nc.vector.tensor_sub(yd[c][:, mg], ya[c][:, mg], yb[c][:, mg])

            # ---- pairs + inverse (diagonal pairing, lane groups of <=4) ----
            yaf = [ya[c][:].rearrange("p a b t -> p (a b t)") for c in range(4)]
            ybf = [yb[c][:].rearrange("p a b t -> p (a b t)") for c in range(4)]
            ysf = [ys[c][:].rearrange("p a b t -> p (a b t)") for c in range(4)]
            ydf = [yd[c][:].rearrange("p a b t -> p (a b t)") for c in range(4)]
            for d in range(1, M):
                lanes = M - d
                kb = sum(M - dd for dd in range(1, d))
                for l0 in range(0, lanes, 2):
                    lc = min(2, lanes - l0)
                    rows = lc * T
                    s1 = slice(l0 * T, l0 * T + rows)            # m1 side
                    s2 = slice((l0 + d) * T, (l0 + d) * T + rows)  # m2 side
                    r_sb = r_pool.tile([128, 12, 2 * T], FP16, tag="ru")
                    for c in range(4):
                        nc.vector.tensor_mul(r_sb[:, 0 + c, :rows], ysf[c][:, s1], yaf[c][:, s2])
                        nc.vector.tensor_mul(r_sb[:, 4 + c, :rows], yaf[c][:, s1], ysf[c][:, s2])
                        if c == 3:
                            nc.vector.tensor_mul(r_sb[:, 8 + c, :rows], ybf[c][:, s1], ydf[c][:, s2])
                        else:
                            nc.gpsimd.tensor_mul(r_sb[:, 8 + c, :rows], ybf[c][:, s1], ydf[c][:, s2])
                    for n0 in range(0, rows, 500):
                        nn = min(500, rows - n0)
                        ps_o = inv_psum.tile([64, 500], F32, tag="ops")
                        for idx in range(12):
                            nc.tensor.matmul(
                                ps_o[:, :nn],
                                g_sb[:, idx],
                                r_sb[:, idx, ds(n0, nn)],
                                start=(idx == 0), stop=(idx == 11),
                            )
                        o_sb = tmp_pool.tile([64, 2, T], F32, tag="osb")
                        nlanes = nn // T
                        nc.scalar.copy(
                            o_sb[:, :nlanes],
                            ps_o[:, :nn].rearrange("p (l t) -> p l t", t=T),
                        )
                        nc.sync.dma_start(
                            out[b, :, ds(kb + l0 + n0 // T, nlanes)],
                            o_sb[:, :nlanes],
                        )
    nc.compile()
    return nc


_NC_CACHE = None


def kernel(x: np.ndarray) -> np.ndarray:
    global _NC_CACHE
    x = np.asarray(x, dtype=np.float32)
    assert x.shape == (B, M, T, L)
    xT = np.ascontiguousarray(x.transpose(0, 1, 3, 2)).astype(np.float16)
    s0 = np.sign(x.sum(axis=-1))  # [B, M, T] DC sign for host PHAT term
    if _NC_CACHE is None:
        _NC_CACHE = build_bass()
    nc = _NC_CACHE
    in_maps = [{"xT": xT[c * NB:(c + 1) * NB]} for c in range(NCORES)]
    trace = bool(int(os.environ.get("GCC_TRACE", "0")))
    res = run_bass_kernel_spmd(nc, in_maps, core_ids=list(range(NCORES)),
                               trace=trace)
    if trace and res.exec_time_ns is not None:
        print(f"HW exec time: {res.exec_time_ns} ns")
        if res.instructions_and_trace is not None:
            print("trace:", res.instructions_and_trace[1])
    out = np.concatenate([r["out"] for r in res.results], axis=0)  # [B,NL,28diag,T]
    plist = [m * (2 * M - m - 1) // 2 + (m + d - m - 1)
             for d in range(1, M) for m in range(M - d)]
    final = np.empty((B, NPAIRS, T, NL), dtype=np.float32)
    final[:, plist] = out.transpose(0, 2, 3, 1)
    # host DC (bin 0) PHAT term: sign(S1)*sign(S2)/L, constant over lags
    i1, i2 = np.triu_indices(M, k=1)
    final += (s0[:, i1] * s0[:, i2])[..., None].astype(np.float32) / L
    return final


# revision 12
# speedup vs baseline: 1.2056x; 1.2056x over previous
"""GCC-PHAT Trainium2 kernel (v7: v2 skeleton + uniform 12-plane inverse).

Pipeline (per core, batch-sharded B=16 -> 2 per core):
  1. Forward rfft as PE matmul in fp16 (fp32 PSUM accumulate):
     xT[b,m,n,t] (host-pretransposed, fp16) @ F[1024,1024] fp16.
     F cols 0..511 = cos(2pi n f/L) f=1..512; cols 512..1022 = -sin, f=1..511;
     col 1023 = zeros (Im X[512] = 0).  Output X.T in PSUM, f on partitions
     (8 chunks of 128: 4 'a' = Re f=128c+r+1, 4 'b' = Im).
  2. PHAT normalize per mic (uniform, no special rows): w' =
     1/sqrt(16*(a^2+b^2)) via ACT Abs_reciprocal_sqrt; ya = a*w', yb = b*w'
     (unit/4) fp16; ys = ya+yb, yd = ya-yb.
     Bin 0 (DC) is handled on the host: PHAT reduces it to
     sign(S1)*sign(S2)/L, constant over lags.  f=512 (chunk3 row127)
     works naturally: its sin rows are identically zero.
  3. Pair products (28 mic pairs, diagonal pairing): Karatsuba
     k1 = ys1*a2, k2 = a1*ys2, k3 = b1*yd2 -> 12 planes fp16
     (DVE: k1,k2,k3[c=3]; Pool: k3[c<3]).
  4. Truncated inverse DFT as PE matmul, G stationary: 12 K-chunks of
     [128f x 64 lags] accumulated into PSUM [64, rows].  G rows carry 16x
     scale + irfft weights/fftshift/slice.
  5. PSUM -> ACT copy -> SBUF -> DMA to out[b, lag, p, t] (lag-major;
     host transposes back and adds the DC term).
  Input DMAs ride the ACT HWDGE queue so the (late-bound) output DMAs on
  sync never head-of-line block the next batch's input.
"""

import os
from contextlib import ExitStack

import numpy as np

import concourse.bass as bass
import concourse.bacc as bacc
import concourse.mybir as mybir
import concourse.tile as tile
from concourse.bass import ds, ts
from concourse.bass_utils import run_bass_kernel_spmd

B, M, T, L = 16, 8, 250, 1024
NCORES = 8
NB = B // NCORES          # batches per core
NPAIRS = (M * (M - 1)) // 2   # 28
NL = 64                   # output lags
F32 = mybir.dt.float32
FP16 = mybir.dt.float16


def _build_F() -> np.ndarray:
    n = np.arange(L, dtype=np.float64)[:, None]
    F = np.zeros((L, L), dtype=np.float64)
    f_a = np.arange(1, 513, dtype=np.float64)[None, :]
    f_b = np.arange(1, 512, dtype=np.float64)[None, :]
    F[:, 0:512] = np.cos(2 * np.pi * n * f_a / L)
    F[:, 512:1023] = -np.sin(2 * np.pi * n * f_b / L)
    F[:, 1023] = 0.0            # Im X[512] slot
    return F.astype(np.float16)


def _build_G() -> np.ndarray:
    """12 inverse planes [128, 64]: idx c = k1, 4+c = k2, 8+c = k3."""
    G = np.zeros((12, 128, NL), dtype=np.float64)
    nj = (np.arange(NL) - 32).astype(np.float64)
    for c in range(4):
        for r in range(128):
            f = 128 * c + r + 1
            w = 1.0 if f == 512 else 2.0
            cosv = 16.0 * w * np.cos(2 * np.pi * f * nj / L) / L
            sinv = 16.0 * w * np.sin(2 * np.pi * f * nj / L) / L
            G[0 + c, r] = cosv - sinv     # k1 = ys1*a2
            G[4 + c, r] = sinv            # k2 = a1*ys2
            G[8 + c, r] = -cosv           # k3 = b1*yd2
    return G.astype(np.float16)


def build_bass() -> bass.Bass:
    nc = bacc.Bacc("TRN2", target_bir_lowering=False, debug=False)
    xT = nc.dram_tensor("xT", [NB, M, L, T], FP16, kind="ExternalInput")
    out = nc.dram_tensor("out", [NB, NL, NPAIRS, T], F32, kind="ExternalOutput")
    Fh = nc.inline_tensor(_build_F(), name="Fmat")
    Gh = nc.inline_tensor(np.ascontiguousarray(_build_G()), name="Gmat")

    with tile.TileContext(nc) as tc, ExitStack() as ctx:
        consts = ctx.enter_context(tc.tile_pool(name="consts", bufs=1))
        xt_pool = ctx.enter_context(tc.tile_pool(name="xt", bufs=3))
        y_pool = ctx.enter_context(tc.tile_pool(name="y", bufs=1))
        tmp_pool = ctx.enter_context(tc.tile_pool(name="tmp", bufs=2))
        r_pool = ctx.enter_context(tc.tile_pool(name="r", bufs=3))
        fwd_psum = ctx.enter_context(tc.tile_pool(name="fps", bufs=3, space="PSUM"))
        inv_psum = ctx.enter_context(tc.tile_pool(name="ips", bufs=2, space="PSUM"))

        f_sb = consts.tile([128, 8, L], FP16)
        fr = Fh[:].rearrange("(k p) c -> p k c", p=128)
        for k in range(8):
            # split so the first matmuls only wait on the first 256KB chunk
            nc.sync.dma_start(f_sb[:, k], fr[:, k])
        g_sb = consts.tile([128, 12, NL], FP16)
        nc.sync.dma_start(g_sb[:], Gh[:].rearrange("i p j -> p i j"))

        for b in range(NB):
            # Y tiles: [128, mg(4), m(2), t] fp16 per (chunk, plane)
            ya = [y_pool.tile([128, 4, 2, T], FP16, tag=f"ya{c}", name=f"ya{c}") for c in range(4)]
            yb = [y_pool.tile([128, 4, 2, T], FP16, tag=f"yb{c}", name=f"yb{c}") for c in range(4)]
            ys = [y_pool.tile([128, 4, 2, T], FP16, tag=f"ys{c}", name=f"ys{c}") for c in range(4)]
            yd = [y_pool.tile([128, 4, 2, T], FP16, tag=f"yd{c}", name=f"yd{c}") for c in range(4)]

            # ---- forward + normalize ----
            for mg in range(4):
                xt_sb = xt_pool.tile([128, 8, 2, T], FP16, tag="xt")
                for mi in range(2):
                    nc.scalar.dma_start(
                        xt_sb[:, :, mi],
                        xT[b, 2 * mg + mi].rearrange("(k p) t -> p k t", p=128),
                    )
                for c in range(4):
                    ps_a = fwd_psum.tile([128, 2, T], F32, tag="psa")
                    ps_b = fwd_psum.tile([128, 2, T], F32, tag="psb")
                    for k in range(8):
                        nc.tensor.matmul(
                            ps_a[:],
                            f_sb[:, k, ts(c, 128)],
                            xt_sb[:, k],
                            start=(k == 0), stop=(k == 7),
                        )
                    for k in range(8):
                        nc.tensor.matmul(
                            ps_b[:],
                            f_sb[:, k, ts(4 + c, 128)],
                            xt_sb[:, k],
                            start=(k == 0), stop=(k == 7),
                        )
                    # normalize (uniform across all rows)
                    sq_a = tmp_pool.tile([128, 2, T], F32, tag="sqa")
                    sq_b = tmp_pool.tile([128, 2, T], F32, tag="sqb")
                    w = tmp_pool.tile([128, 2, T], F32, tag="w")
                    nc.scalar.square(sq_a[:], ps_a[:])
                    nc.scalar.square(sq_b[:], ps_b[:])
                    nc.gpsimd.tensor_add(sq_a[:], sq_a[:], sq_b[:])
                    # w' = 1/sqrt(16*r) = (1/|X|)/4
                    nc.scalar.activation(
                        w[:], sq_a[:],
                        mybir.ActivationFunctionType.Abs_reciprocal_sqrt,
                        scale=16.0,
                    )
                    nc.vector.tensor_mul(ya[c][:, mg], ps_a[:], w[:])
                    nc.vector.tensor_mul(yb[c][:, mg], ps_b[:], w[:])
                    nc.vector.tensor_add(ys[c][:, mg], ya[c][:, mg], yb[c][:, mg])
                    nc.vector.tensor_sub(yd[c][:, mg], ya[c][:, mg], yb[c][:, mg])

            # ---- pairs + inverse (diagonal pairing, lane groups of <=4) ----
            yaf = [ya[c][:].rearrange("p a b t -> p (a b t)") for c in range(4)]
            ybf = [yb[c][:].rearrange("p a b t -> p (a b t)") for c in range(4)]
            ysf = [ys[c][:].rearrange("p a b t -> p (a b t)") for c in range(4)]
            ydf = [yd[c][:].rearrange("p a b t -> p (a b t)") for c in range(4)]
            for d in range(1, M):
                lanes = M - d
                kb = sum(M - dd for dd in range(1, d))
                for l0 in range(0, lanes, 4):
                    lc = min(4, lanes - l0)
                    rows = lc * T
                    s1 = slice(l0 * T, l0 * T + rows)            # m1 side
                    s2 = slice((l0 + d) * T, (l0 + d) * T + rows)  # m2 side
                    r_sb = r_pool.tile([128, 12, 4 * T], FP16, tag="ru")
                    for c in range(4):
                        nc.vector.tensor_mul(r_sb[:, 0 + c, :rows], ysf[c][:, s1], yaf[c][:, s2])
                        nc.vector.tensor_mul(r_sb[:, 4 + c, :rows], yaf[c][:, s1], ysf[c][:, s2])
                        nc.gpsimd.tensor_mul(r_sb[:, 8 + c, :rows], ybf[c][:, s1], ydf[c][:, s2])
                    for n0 in range(0, rows, 500):
                        nn = min(500, rows - n0)
                        ps_o = inv_psum.tile([64, 500], F32, tag="ops")
                        for idx in range(12):
                            nc.tensor.matmul(
                                ps_o[:, :nn],
                                g_sb[:, idx],
                                r_sb[:, idx, ds(n0, nn)],
                                start=(idx == 0), stop=(idx == 11),
                            )
                        o_sb = tmp_pool.tile([64, 2, T], F32, tag="osb")
                        nlanes = nn // T
                        nc.scalar.copy(
                            o_sb[:, :nlanes],
                            ps_o[:, :nn].rearrange("p (l t) -> p l t", t=T),
                        )
                        nc.sync.dma_start(
                            out[b, :, ds(kb + l0 + n0 // T, nlanes)],
                            o_sb[:, :nlanes],
                        )
    nc.compile()
    return nc


_NC_CACHE = None


def kernel(x: np.ndarray) -> np.ndarray:
    global _NC_CACHE
    x = np.asarray(x, dtype=np.float32)
    assert x.shape == (B, M, T, L)
    xT = np.ascontiguousarray(x.transpose(0, 1, 3, 2)).astype(np.float16)
    s0 = np.sign(x.sum(axis=-1))  # [B, M, T] DC sign for host PHAT term
    if _NC_CACHE is None:
        _NC_CACHE = build_bass()
    nc = _NC_CACHE
    in_maps = [{"xT": xT[c * NB:(c + 1) * NB]} for c in range(NCORES)]
    trace = bool(int(os.environ.get("GCC_TRACE", "0")))
    res = run_bass_kernel_spmd(nc, in_maps, core_ids=list(range(NCORES)),
                               trace=trace)
    if trace and res.exec_time_ns is not None:
        print(f"HW exec time: {res.exec_time_ns} ns")
        if res.instructions_and_trace is not None:
            print("trace:", res.instructions_and_trace[1])
    out = np.concatenate([r["out"] for r in res.results], axis=0)  # [B,NL,28diag,T]
    plist = [m * (2 * M - m - 1) // 2 + (m + d - m - 1)
             for d in range(1, M) for m in range(M - d)]
    final = np.empty((B, NPAIRS, T, NL), dtype=np.float32)
    final[:, plist] = out.transpose(0, 2, 3, 1)
    # host DC (bin 0) PHAT term: sign(S1)*sign(S2)/L, constant over lags
    i1, i2 = np.triu_indices(M, k=1)
    final += (s0[:, i1] * s0[:, i2])[..., None].astype(np.float32) / L
    return final


# revision 13
# speedup vs baseline: 1.2611x; 1.0460x over previous
"""GCC-PHAT Trainium2 kernel (v7: v2 skeleton + uniform 12-plane inverse).

Pipeline (per core, batch-sharded B=16 -> 2 per core):
  1. Forward rfft as PE matmul in fp16 (fp32 PSUM accumulate):
     xT[b,m,n,t] (host-pretransposed, fp16) @ F[1024,1024] fp16.
     F cols 0..511 = cos(2pi n f/L) f=1..512; cols 512..1022 = -sin, f=1..511;
     col 1023 = zeros (Im X[512] = 0).  Output X.T in PSUM, f on partitions
     (8 chunks of 128: 4 'a' = Re f=128c+r+1, 4 'b' = Im).
  2. PHAT normalize per mic (uniform, no special rows): w' =
     1/sqrt(16*(a^2+b^2)) via ACT Abs_reciprocal_sqrt; ya = a*w', yb = b*w'
     (unit/4) fp16; ys = ya+yb, yd = ya-yb.
     Bin 0 (DC) is handled on the host: PHAT reduces it to
     sign(S1)*sign(S2)/L, constant over lags.  f=512 (chunk3 row127)
     works naturally: its sin rows are identically zero.
  3. Pair products (28 mic pairs, diagonal pairing): Karatsuba
     k1 = ys1*a2, k2 = a1*ys2, k3 = b1*yd2 -> 12 planes fp16
     (DVE: k1,k2,k3[c=3]; Pool: k3[c<3]).
  4. Truncated inverse DFT as PE matmul, G stationary: 12 K-chunks of
     [128f x 64 lags] accumulated into PSUM [64, rows].  G rows carry 16x
     scale + irfft weights/fftshift/slice.
  5. PSUM -> ACT copy -> SBUF -> DMA to out[b, lag, p, t] (lag-major;
     host transposes back and adds the DC term).
  Input DMAs ride the ACT HWDGE queue so the (late-bound) output DMAs on
  sync never head-of-line block the next batch's input.
"""

import os
from contextlib import ExitStack

import numpy as np

import concourse.bass as bass
import concourse.bacc as bacc
import concourse.mybir as mybir
import concourse.tile as tile
from concourse.bass import ds, ts
from concourse.bass_utils import run_bass_kernel_spmd

B, M, T, L = 16, 8, 250, 1024
NCORES = 8
NB = B // NCORES          # batches per core
NPAIRS = (M * (M - 1)) // 2   # 28
NL = 64                   # output lags
F32 = mybir.dt.float32
FP16 = mybir.dt.float16


def _build_F() -> np.ndarray:
    n = np.arange(L, dtype=np.float64)[:, None]
    F = np.zeros((L, L), dtype=np.float64)
    f_a = np.arange(1, 513, dtype=np.float64)[None, :]
    f_b = np.arange(1, 512, dtype=np.float64)[None, :]
    F[:, 0:512] = np.cos(2 * np.pi * n * f_a / L)
    F[:, 512:1023] = -np.sin(2 * np.pi * n * f_b / L)
    F[:, 1023] = 0.0            # Im X[512] slot
    return F.astype(np.float16)


def _build_G() -> np.ndarray:
    """12 inverse planes [128, 64]: idx c = k1, 4+c = k2, 8+c = k3."""
    G = np.zeros((12, 128, NL), dtype=np.float64)
    nj = (np.arange(NL) - 32).astype(np.float64)
    for c in range(4):
        for r in range(128):
            f = 128 * c + r + 1
            w = 1.0 if f == 512 else 2.0
            cosv = 16.0 * w * np.cos(2 * np.pi * f * nj / L) / L
            sinv = 16.0 * w * np.sin(2 * np.pi * f * nj / L) / L
            G[0 + c, r] = cosv - sinv     # k1 = ys1*a2
            G[4 + c, r] = sinv            # k2 = a1*ys2
            G[8 + c, r] = -cosv           # k3 = b1*yd2
    return G.astype(np.float16)


def build_bass() -> bass.Bass:
    nc = bacc.Bacc("TRN2", target_bir_lowering=False, debug=False)
    xT = nc.dram_tensor("xT", [NB, M, L, T], FP16, kind="ExternalInput")
    out = nc.dram_tensor("out", [NB, NL, NPAIRS, T], F32, kind="ExternalOutput")
    Fh = nc.inline_tensor(_build_F(), name="Fmat")
    Gh = nc.inline_tensor(np.ascontiguousarray(_build_G()), name="Gmat")

    with tile.TileContext(nc) as tc, ExitStack() as ctx:
        consts = ctx.enter_context(tc.tile_pool(name="consts", bufs=1))
        xt_pool = ctx.enter_context(tc.tile_pool(name="xt", bufs=3))
        y_pool = ctx.enter_context(tc.tile_pool(name="y", bufs=1))
        tmp_pool = ctx.enter_context(tc.tile_pool(name="tmp", bufs=2))
        r_pool = ctx.enter_context(tc.tile_pool(name="r", bufs=3))
        fwd_psum = ctx.enter_context(tc.tile_pool(name="fps", bufs=3, space="PSUM"))
        inv_psum = ctx.enter_context(tc.tile_pool(name="ips", bufs=2, space="PSUM"))

        f_sb = consts.tile([128, 8, L], FP16)
        fr = Fh[:].rearrange("(k p) c -> p k c", p=128)
        for k in range(8):
            # split so the first matmuls only wait on the first 256KB chunk
            nc.sync.dma_start(f_sb[:, k], fr[:, k])
        g_sb = consts.tile([128, 12, NL], FP16)
        nc.sync.dma_start(g_sb[:], Gh[:].rearrange("i p j -> p i j"))

        for b in range(NB):
            # Y tiles: [128, mg(4), m(2), t] fp16 per (chunk, plane)
            ya = [y_pool.tile([128, 4, 2, T], FP16, tag=f"ya{c}", name=f"ya{c}") for c in range(4)]
            yb = [y_pool.tile([128, 4, 2, T], FP16, tag=f"yb{c}", name=f"yb{c}") for c in range(4)]
            ys = [y_pool.tile([128, 4, 2, T], FP16, tag=f"ys{c}", name=f"ys{c}") for c in range(4)]
            yd = [y_pool.tile([128, 4, 2, T], FP16, tag=f"yd{c}", name=f"yd{c}") for c in range(4)]

            # ---- forward + normalize ----
            for mg in range(4):
                xt_sb = xt_pool.tile([128, 8, 2, T], FP16, tag="xt")
                for mi in range(2):
                    nc.scalar.dma_start(
                        xt_sb[:, :, mi],
                        xT[b, 2 * mg + mi].rearrange("(k p) t -> p k t", p=128),
                    )
                for c in range(4):
                    ps_a = fwd_psum.tile([128, 2, T], F32, tag="psa")
                    ps_b = fwd_psum.tile([128, 2, T], F32, tag="psb")
                    for k in range(8):
                        nc.tensor.matmul(
                            ps_a[:],
                            f_sb[:, k, ts(c, 128)],
                            xt_sb[:, k],
                            start=(k == 0), stop=(k == 7),
                        )
                    for k in range(8):
                        nc.tensor.matmul(
                            ps_b[:],
                            f_sb[:, k, ts(4 + c, 128)],
                            xt_sb[:, k],
                            start=(k == 0), stop=(k == 7),
                        )
                    # normalize (uniform across all rows)
                    sq_a = tmp_pool.tile([128, 2, T], F32, tag="sqa")
                    sq_b = tmp_pool.tile([128, 2, T], F32, tag="sqb")
                    w = tmp_pool.tile([128, 2, T], F32, tag="w")
                    nc.scalar.square(sq_a[:], ps_a[:])
                    nc.scalar.square(sq_b[:], ps_b[:])
                    nc.gpsimd.tensor_add(sq_a[:], sq_a[:], sq_b[:])
                    # w' = 1/sqrt(16*r) = (1/|X|)/4
                    nc.scalar.activation(
                        w[:], sq_a[:],
                        mybir.ActivationFunctionType.Abs_reciprocal_sqrt,
                        scale=16.0,
                    )
                    nc.vector.tensor_mul(ya[c][:, mg], ps_a[:], w[:])
                    nc.vector.tensor_mul(yb[c][:, mg], ps_b[:], w[:])
                    nc.vector.tensor_add(ys[c][:, mg], ya[c][:, mg], yb[c][:, mg])
                    nc.vector.tensor_sub(yd[c][:, mg], ya[c][:, mg], yb[c][:, mg])

            # ---- pairs + inverse (diagonal pairing, lane groups of <=4) ----
            yaf = [ya[c][:].rearrange("p a b t -> p (a b t)") for c in range(4)]
            ybf = [yb[c][:].rearrange("p a b t -> p (a b t)") for c in range(4)]
            ysf = [ys[c][:].rearrange("p a b t -> p (a b t)") for c in range(4)]
            ydf = [yd[c][:].rearrange("p a b t -> p (a b t)") for c in range(4)]
            for d in range(1, M):
                lanes = M - d
                kb = sum(M - dd for dd in range(1, d))
                for l0 in range(0, lanes, 4):
                    lc = min(4, lanes - l0)
                    rows = lc * T
                    s1 = slice(l0 * T, l0 * T + rows)            # m1 side
                    s2 = slice((l0 + d) * T, (l0 + d) * T + rows)  # m2 side
                    r_sb = r_pool.tile([128, 12, 4 * T], FP16, tag="ru")
                    for c in range(4):
                        nc.vector.tensor_mul(r_sb[:, 0 + c, :rows], ysf[c][:, s1], yaf[c][:, s2])
                        nc.vector.tensor_mul(r_sb[:, 4 + c, :rows], yaf[c][:, s1], ysf[c][:, s2])
                        if c == 3:
                            nc.vector.tensor_mul(r_sb[:, 8 + c, :rows], ybf[c][:, s1], ydf[c][:, s2])
                        else:
                            nc.gpsimd.tensor_mul(r_sb[:, 8 + c, :rows], ybf[c][:, s1], ydf[c][:, s2])
                    for n0 in range(0, rows, 500):
                        nn = min(500, rows - n0)
                        ps_o = inv_psum.tile([64, 500], F32, tag="ops")
                        for idx in range(12):
                            nc.tensor.matmul(
                                ps_o[:, :nn],
                                g_sb[:, idx],
                                r_sb[:, idx, ds(n0, nn)],
                                start=(idx == 0), stop=(idx == 11),
                            )
                        o_sb = tmp_pool.tile([64, 2, T], F32, tag="osb")
                        nlanes = nn // T
                        nc.scalar.copy(
                            o_sb[:, :nlanes],
                            ps_o[:, :nn].rearrange("p (l t) -> p l t", t=T),
                        )
                        nc.sync.dma_start(
                            out[b, :, ds(kb + l0 + n0 // T, nlanes)],
                            o_sb[:, :nlanes],
                        )
    nc.compile()
    return nc


_NC_CACHE = None


def kernel(x: np.ndarray) -> np.ndarray:
    global _NC_CACHE
    x = np.asarray(x, dtype=np.float32)
    assert x.shape == (B, M, T, L)
    xT = np.ascontiguousarray(x.transpose(0, 1, 3, 2)).astype(np.float16)
    s0 = np.sign(x.sum(axis=-1))  # [B, M, T] DC sign for host PHAT term
    if _NC_CACHE is None:
        _NC_CACHE = build_bass()
    nc = _NC_CACHE
    in_maps = [{"xT": xT[c * NB:(c + 1) * NB]} for c in range(NCORES)]
    trace = bool(int(os.environ.get("GCC_TRACE", "0")))
    res = run_bass_kernel_spmd(nc, in_maps, core_ids=list(range(NCORES)),
                               trace=trace)
    if trace and res.exec_time_ns is not None:
        print(f"HW exec time: {res.exec_time_ns} ns")
        if res.instructions_and_trace is not None:
            print("trace:", res.instructions_and_trace[1])
    out = np.concatenate([r["out"] for r in res.results], axis=0)  # [B,NL,28diag,T]
    plist = [m * (2 * M - m - 1) // 2 + (m + d - m - 1)
             for d in range(1, M) for m in range(M - d)]
    final = np.empty((B, NPAIRS, T, NL), dtype=np.float32)
    final[:, plist] = out.transpose(0, 2, 3, 1)
    # host DC (bin 0) PHAT term: sign(S1)*sign(S2)/L, constant over lags
    i1, i2 = np.triu_indices(M, k=1)
    final += (s0[:, i1] * s0[:, i2])[..., None].astype(np.float32) / L
    return final
